# revision 7
# baseline (speedup 1.0000x reference)
"""Trainium2 Bass kernel for nn_DecoderBlock (dynamic-conv decoder block).

v3: NO collective. Pure data-parallel over batch (2 samples/core); the
kernel-predictor weights (kpsw [4609, 4096] bf16, ~38 MB) are replicated
and streamed once per core, and each core computes all 4096 predictor
columns for only its OWN 2 samples. This removes the AllToAll that
previously serialized core 0 against the staggered per-core NEFF
launches (~9 ms apart -> ~70 ms stall on the profiled core).

Math per sample (C=512, G=64, cg=8, H=W=32, S=512, Cout=256):
  dw   = conv3x3(reflect_pad(w), kp_sw) + kp_sb        # kernel predictor
  pw   = pooled @ kp_pw.T + kp_pb ;  bias = pooled @ kp_bw.T + kp_bb
  xn   = instance_norm(x)
  y    = grouped_dynconv3x3(reflect_pad(xn), dw)       # per-sample weights
  y    = grouped_pointwise(pw, y) + bias
  y    = relu(conv3x3(y, dec_w1) + b1)
  y    = relu(conv3x3(y, dec_w2) + b2)
  out  = nearest_upsample_2x(y)

The predictor matmuls accumulate [128-col-chunk, (b, opos)] regions in two
PSUM banks over the (9 kpos x 4 s-chunk) contraction; results land in
dwTs [128, 32, b, tap] and are DMA'd to a DRAM scratch in the same slab
layout the old AllToAll produced, so the dynamic-conv phase (block-diag
[128,128] weight tiles built with identity-selector matmuls) is unchanged.

Queue plan: kpsw/kppw stream alternates sync/gpsimd; x loads on the DVE
queue; small predictor/bias weights + the identity build on ACT's queue;
w1t on gpsimd and w2t on ACT (needed only by phases B/C, so they drain
after the predictor stream).
"""

import sys

sys.path.insert(0, "/opt/trn_rl_repo")

import numpy as np
import ml_dtypes

import concourse.bacc as bacc
import concourse.tile as tile
from concourse import mybir
from concourse.alu_op_type import AluOpType
from concourse.bass_utils import run_bass_kernel_spmd

F32 = mybir.dt.float32
BF16 = mybir.dt.bfloat16
AF = mybir.ActivationFunctionType

NCORES = 8
B = 16           # total batch
BPC = 2          # samples per core
C = 512          # in channels
CO = 256         # out channels
S = 512          # style dim
G = 64           # groups
CG = 8           # channels per group
H = W = 32
HW = H * W
NT = C // 128    # 4 channel tiles
NM2 = CO // 128  # 2 out-channel tiles
EPS = 1e-5
SCR = 128 * 129  # 16512; span of the diag-embedded identity build
NCOL = C * CG    # 4096 predictor columns (all on every core now)
NCC = NCOL // 128  # 32 column chunks
NTAP = 10        # 9 dw taps + the pw tap
CCW = NTAP * 512  # 5120 = per-slab width in the dw scratch buffer
BPO = BPC * 9    # 18 = (own sample, out-position) columns in predictor matmul

_CACHE = {}


def _build():
    nc = bacc.Bacc(None, target_bir_lowering=False)

    x2 = nc.declare_dram_parameter("x2", [BPC, C, H, W], BF16, isOutput=False)
    wownr = nc.declare_dram_parameter("wownr", [128, 4, BPC, 9], F32, isOutput=False)
    kpsws = nc.declare_dram_parameter("kpsws", [9 * S + 1, NCOL], BF16, isOutput=False)
    kppws = nc.declare_dram_parameter("kppws", [S + 1, NCOL], BF16, isOutput=False)
    kpbw = nc.declare_dram_parameter("kpbw", [S + 1, C], BF16, isOutput=False)
    w1t = nc.declare_dram_parameter("w1t", [9, C, C], BF16, isOutput=False)
    w2t = nc.declare_dram_parameter("w2t", [9, C, CO], BF16, isOutput=False)
    b1d = nc.declare_dram_parameter("b1d", [C], F32, isOutput=False)
    b2d = nc.declare_dram_parameter("b2d", [CO], F32, isOutput=False)
    yout = nc.declare_dram_parameter("yout", [BPC, CO, 2 * H, 2 * W], F32, isOutput=True)

    ccout = nc.dram_tensor("ccout", [B, CCW], BF16)
    idscr = nc.dram_tensor("idscr", [SCR], BF16)

    with tile.TileContext(nc) as tc:
        with (
            tc.tile_pool(name="consts", bufs=1) as consts,
            tc.tile_pool(name="kstream", bufs=4) as kstream,
            tc.tile_pool(name="wres", bufs=1) as wres,
            tc.tile_pool(name="dwp", bufs=1) as dwpp,
            tc.tile_pool(name="blk", bufs=4) as blkp,
            tc.tile_pool(name="act", bufs=3) as actp,
            tc.tile_pool(name="pad3", bufs=4) as pad3,
            tc.tile_pool(name="outp", bufs=4) as outp,
            tc.tile_pool(name="psum", bufs=8, space="PSUM") as psum,
        ):
            # ---------------- persistent small constants ----------------
            epsb = consts.tile([128, 1], F32, tag="epsb")
            nc.vector.memset(epsb[:], EPS)
            # prime the ACT function table while the first DMAs run
            actwarm = consts.tile([128, 1], F32, tag="actwarm")
            nc.scalar.mul(actwarm[:], epsb[:], 1.0)
            onesf = consts.tile([1, BPO], F32, tag="onesf")
            nc.vector.memset(onesf[:], 1.0)
            ones = consts.tile([1, BPO], BF16, tag="ones")
            nc.vector.tensor_copy(ones[:], onesf[:])
            zb16 = consts.tile([128, 132], BF16, tag="zb16")
            nc.vector.memset(zb16[:], 0.0)
            onescol = consts.tile([128, 1], BF16, tag="onescol")
            nc.vector.memset(onescol[:], 1.0)

            # ---------------- own style maps + pooled ----------------
            wosb = consts.tile([128, 4, BPC, 9], F32, tag="wosb")
            nc.sync.dma_start(out=wosb[:], in_=wownr[:, :, :, :])
            # bias-row and small-weight loads, all on queues that the big
            # kpsw stream does not use
            rb = consts.tile([1, NCOL], BF16, tag="rb")
            nc.scalar.dma_start(out=rb[:], in_=kpsws[9 * S : 9 * S + 1, :])
            rbp = consts.tile([1, NCOL], BF16, tag="rbp")
            nc.scalar.dma_start(out=rbp[:], in_=kppws[S : S + 1, :])
            b1sb = consts.tile([128, NT], F32, tag="b1sb")
            nc.scalar.dma_start(out=b1sb[:, :], in_=b1d.rearrange("(m c) -> c m", c=128))
            b2sb = consts.tile([128, NM2], F32, tag="b2sb")
            nc.scalar.dma_start(out=b2sb[:, :], in_=b2d.rearrange("(m c) -> c m", c=128))
            kbsb = consts.tile([128, 4, C], BF16, tag="kbsb")
            nc.scalar.dma_start(
                out=kbsb[:], in_=kpbw[:S, :].rearrange("(q s) c -> s q c", q=4)
            )
            rbb = consts.tile([1, C], BF16, tag="rbb")
            nc.scalar.dma_start(out=rbb[:], in_=kpbw[S : S + 1, :])

            # identity for the block-diag build: idT[i, g, m] = (m == 8g+i),
            # via a DRAM stride-129 diagonal embedding
            nc.scalar.dma_start(
                out=idscr.rearrange("(p c) -> p c", c=129), in_=zb16[:, :129]
            )
            nc.scalar.dma_start(
                out=idscr.rearrange("(p c) -> p c", c=129)[:, :1], in_=onescol[:]
            )
            idT = consts.tile([8, 16, 128], BF16, tag="idT")
            nc.scalar.dma_start(
                out=idT[:],
                in_=idscr[: 128 * 128]
                .rearrange("(gi m) -> gi m", m=128)
                .rearrange("(g i) m -> i g m", g=16),
            )

            # reflect-pad 3x3 -> 5x5 (batched over b), bf16; on ACT to keep
            # DVE free for the im2col below
            wp = consts.tile([128, 4, BPC, 5, 5], BF16, tag="wp")
            for q in range(4):
                w3 = wosb[:, q, :, :].rearrange("p b (kh kw) -> p b kh kw", kh=3)
                if q < 2:
                    nc.scalar.mul(wp[:, q, :, 1:4, 1:4], w3, 1.0)
                    nc.scalar.mul(wp[:, q, :, 1:4, 0:1], w3[:, :, :, 1:2], 1.0)
                    nc.scalar.mul(wp[:, q, :, 1:4, 4:5], w3[:, :, :, 1:2], 1.0)
                    nc.scalar.mul(wp[:, q, :, 0, :], wp[:, q, :, 2, :], 1.0)
                    nc.scalar.mul(wp[:, q, :, 4, :], wp[:, q, :, 2, :], 1.0)
                else:
                    nc.vector.tensor_copy(wp[:, q, :, 1:4, 1:4], w3)
                    nc.vector.tensor_copy(wp[:, q, :, 1:4, 0:1], w3[:, :, :, 1:2])
                    nc.vector.tensor_copy(wp[:, q, :, 1:4, 4:5], w3[:, :, :, 1:2])
                    nc.vector.tensor_copy(wp[:, q, :, 0, :], wp[:, q, :, 2, :])
                    nc.vector.tensor_copy(wp[:, q, :, 4, :], wp[:, q, :, 2, :])

            # im2col of padded style map: [s-chunk, kpos, q, (b, opos)]
            xw = consts.tile([128, 9, 4, BPO], BF16, tag="xw")
            for di in range(3):
                for dj in range(3):
                    nc.vector.tensor_copy(
                        xw[:, di * 3 + dj, :, :].rearrange(
                            "p q (b i j) -> p q b i j", b=BPC, i=3
                        ),
                        wp[:, :, :, di : di + 3, dj : dj + 3],
                    )

            pooledo_f = consts.tile([128, 4, BPC], F32, tag="pooledof")
            for q in range(4):
                nc.vector.tensor_reduce(
                    out=pooledo_f[:, q, :],
                    in_=wosb[:, q, :, :],
                    axis=mybir.AxisListType.X,
                    op=AluOpType.add,
                )
            pooledo = consts.tile([128, 4, BPC], BF16, tag="pooledo")
            nc.scalar.mul(pooledo[:, :, :], pooledo_f[:, :, :], 1.0 / 9.0)

            # instance norm -> reflect-padded xn (bf16); x loads ride the
            # DVE queue so they overlap the predictor weight stream
            xps = []
            for b in range(BPC):
                xp = pad3.tile([128, NT, 34, 34], BF16, tag="padbuf")
                xps.append(xp)
                for t in range(NT):
                    xsb = actp.tile([128, HW], BF16, tag="xsb")
                    nc.scalar.dma_start(
                        out=xsb[:],
                        in_=x2[b, 128 * t : 128 * (t + 1), :, :].rearrange(
                            "c h w -> c (h w)"
                        ),
                    )
                    st = actp.tile([128, 2, 6], F32, tag="bnst")
                    xsb2 = xsb[:].rearrange("p (s f) -> p s f", f=512)
                    for sg in range(2):
                        nc.vector.bn_stats(out=st[:, sg, :], in_=xsb2[:, sg, :])
                    mv = actp.tile([128, 2], F32, tag="bnmv")
                    nc.vector.bn_aggr(out=mv[:], in_=st[:])
                    rstd = actp.tile([128, 1], F32, tag="rstd")
                    nc.scalar.activation(
                        out=rstd[:], in_=mv[:, 1:2], func=AF.Sqrt, bias=epsb[:], scale=1.0
                    )
                    nc.vector.reciprocal(out=rstd[:], in_=rstd[:])
                    nc.vector.tensor_scalar(
                        out=xp[:, t, 1:33, 1:33],
                        in0=xsb[:].rearrange("p (h w) -> p h w", h=H),
                        scalar1=mv[:, 0:1],
                        scalar2=rstd[:],
                        op0=AluOpType.subtract,
                        op1=AluOpType.mult,
                    )
                    nc.vector.tensor_copy(xp[:, t, 1:33, 0:1], xp[:, t, 1:33, 2:3])
                    nc.vector.tensor_copy(xp[:, t, 1:33, 33:34], xp[:, t, 1:33, 31:32])
                    nc.vector.tensor_copy(xp[:, t, 0, :], xp[:, t, 2, :])
                    nc.vector.tensor_copy(xp[:, t, 33, :], xp[:, t, 31, :])

            # ------- predictor: dw cols (ALL 32 chunks, own samples) -------
            # psum regions: pd0 holds chunks 0..27 at [18cc, 18cc+18);
            # pd1 holds chunks 28..31 then the 32 pw 2-wide regions at 72+.
            pd0 = psum.tile([128, 512], F32, tag="mm", name="pd0")
            pd1 = psum.tile([128, 512], F32, tag="mm", name="pd1")

            def _dwreg(cc):
                if cc < 28:
                    return pd0[:, 18 * cc : 18 * cc + 18]
                return pd1[:, 18 * (cc - 28) : 18 * (cc - 28) + 18]

            def _pwreg(cc):
                return pd1[:, 72 + 2 * cc : 74 + 2 * cc]

            # PSUM "start" marks the whole 2 KB bank zero-on-next-write, so
            # each bank gets exactly ONE start (its first matmul) and ONE
            # stop (its last); every chain's first write then lands on the
            # zeroed bank and later writes accumulate.
            for kpos in range(9):
                for ch in range(8):
                    ksb = kstream.tile([128, 4, 512], BF16, tag="ksb")
                    dma = nc.sync.dma_start if (kpos + ch) % 2 else nc.gpsimd.dma_start
                    dma(
                        out=ksb[:],
                        in_=kpsws[
                            kpos * 512 : (kpos + 1) * 512, 512 * ch : 512 * (ch + 1)
                        ].rearrange("(q s) c -> s q c", q=4),
                    )
                    for q in range(4):
                        for sub in range(4):
                            cc = 4 * ch + sub
                            nc.tensor.matmul(
                                _dwreg(cc),
                                ksb[:, q, 128 * sub : 128 * (sub + 1)],
                                xw[:, kpos, q, :],
                                start=(kpos == 0 and q == 0 and cc in (0, 28)),
                                stop=False,
                            )
            # pw tap: pooled @ kp_pw.T, streamed over the same col chunks
            for ch in range(8):
                kpb_t = kstream.tile([128, 4, 512], BF16, tag="ksb")
                dma = nc.sync.dma_start if ch % 2 else nc.gpsimd.dma_start
                dma(
                    out=kpb_t[:],
                    in_=kppws[:S, 512 * ch : 512 * (ch + 1)].rearrange(
                        "(q s) c -> s q c", q=4
                    ),
                )
                for q in range(4):
                    for sub in range(4):
                        cc = 4 * ch + sub
                        nc.tensor.matmul(
                            _pwreg(cc),
                            kpb_t[:, q, 128 * sub : 128 * (sub + 1)],
                            pooledo[:, q, :],
                            start=False,
                            stop=False,
                        )
            # bias rows close every accumulation chain; stop only on each
            # bank's final matmul (pd0: dw cc=27, pd1: pw cc=31)
            for cc in range(NCC):
                nc.tensor.matmul(
                    _dwreg(cc),
                    rb[:1, 128 * cc : 128 * (cc + 1)],
                    ones[:1, :BPO],
                    start=False,
                    stop=(cc == 27),
                )
            for cc in range(NCC):
                nc.tensor.matmul(
                    _pwreg(cc),
                    rbp[:1, 128 * cc : 128 * (cc + 1)],
                    ones[:1, :BPC],
                    start=False,
                    stop=(cc == 31),
                )

            # dwTs[p, cc, b, tap]: taps 0..8 = dw out positions, tap 9 = pw
            dwTs = consts.tile([128, NCC, BPC, NTAP], BF16, tag="dwTs")
            for cc in range(NCC):
                if cc % 2:
                    nc.scalar.mul(
                        dwTs[:, cc, :, :9],
                        _dwreg(cc).rearrange("p (b pos) -> p b pos", b=BPC),
                        1.0,
                    )
                    nc.scalar.mul(dwTs[:, cc, :, 9], _pwreg(cc), 1.0)
                else:
                    nc.vector.tensor_copy(
                        dwTs[:, cc, :, :9],
                        _dwreg(cc).rearrange("p (b pos) -> p b pos", b=BPC),
                    )
                    nc.vector.tensor_copy(dwTs[:, cc, :, 9], _pwreg(cc))

            # scatter to the DRAM scratch in the slab layout phase A reads:
            # slab 2*(2t+gh)+b, within-slab index = (128*blk + p)*10 + tap,
            # where cc = 8t + 4gh + blk.
            for t in range(NT):
                for gh in range(2):
                    for b in range(BPC):
                        nc.sync.dma_start(
                            out=ccout.rearrange(
                                "(s2 b) (blk p pos) -> s2 b p blk pos",
                                b=BPC, blk=4, pos=NTAP,
                            )[2 * t + gh, b],
                            in_=dwTs[:, 8 * t + 4 * gh : 8 * t + 4 * gh + 4, b, :],
                        )

            # ------- bias predictor (own 2 samples) ---
            biasc = consts.tile([128, NT, BPC], F32, tag="biasc")
            for m in range(NT):
                ps3 = psum.tile([128, 512], F32, tag="mm")
                for q in range(4):
                    nc.tensor.matmul(
                        ps3[:, :BPC],
                        kbsb[:, q, 128 * m : 128 * (m + 1)],
                        pooledo[:, q, :],
                        start=(q == 0),
                        stop=False,
                    )
                nc.tensor.matmul(
                    ps3[:, :BPC],
                    rbb[:1, 128 * m : 128 * (m + 1)],
                    ones[:1, :BPC],
                    start=False,
                    stop=True,
                )
                nc.vector.tensor_copy(biasc[:, m, :], ps3[:, :BPC])

            # decoder conv weights, SBUF-resident for the whole kernel;
            # queued behind the predictor stream (gpsimd) / small loads
            # (scalar) so they drain in the phase-A window
            w1sb = []
            for k in range(NT):
                t_ = wres.tile([128, 9, C], BF16, tag=f"w1k{k}")
                w1sb.append(t_)
                for p3 in range(3):
                    nc.gpsimd.dma_start(
                        out=t_[:, 3 * p3 : 3 * (p3 + 1), :],
                        in_=w1t[3 * p3 : 3 * (p3 + 1), 128 * k : 128 * (k + 1), :]
                        .rearrange("pos p co -> p pos co"),
                    )
            w2sb = []
            for k in range(NT):
                t_ = wres.tile([128, 9, CO], BF16, tag=f"w2k{k}")
                w2sb.append(t_)
                nc.scalar.dma_start(
                    out=t_[:],
                    in_=w2t[:, 128 * k : 128 * (k + 1), :].rearrange(
                        "pos p co -> p pos co"
                    ),
                )

            # zero-padded output buffers for conv1
            def _zero_border(yp):
                for t in range(NT):
                    nc.vector.tensor_copy(yp[:, t, 0, :], zb16[:, :34])
                    nc.vector.tensor_copy(yp[:, t, 33, :], zb16[:, :34])
                    nc.vector.tensor_copy(
                        yp[:, t, 1:33, 0:1],
                        zb16[:, :32].rearrange("p (a c) -> p a c", c=1),
                    )
                    nc.vector.tensor_copy(
                        yp[:, t, 1:33, 33:34],
                        zb16[:, :32].rearrange("p (a c) -> p a c", c=1),
                    )

            yp1s = []
            for b in range(BPC):
                yp1 = pad3.tile([128, NT, 34, 34], BF16, tag="padbuf")
                yp1s.append(yp1)
                _zero_border(yp1)

            # ---------------- phase A: adaconv (dynamic grouped conv) --------
            # dsrcc[i, gh, gl, (co tap)] <- ccout. Expand to block-diag
            # [128,128] tiles with identity-selector matmuls: lhsT idT[:,g,:]
            # routes source row i to partition 8g+i and writes zeros to all
            # other partitions, so one matmul per group builds its 8x8 block
            # across all 10 taps at once.
            GRPS = ((0, 6), (6, 6), (12, 4))  # psum passes over the 16 groups
            dwpws = {}
            for b in range(BPC):
                for t in range(NT):
                    dsrcc = blkp.tile([8, 2, 8, 8 * NTAP], BF16, tag="dsrcc")
                    for gh in range(2):
                        nc.sync.dma_start(
                            out=dsrcc[:, gh, :, :],
                            in_=ccout[2 * (2 * t + gh) + b, :].rearrange(
                                "(gl i co tap) -> i gl (co tap)", gl=8, i=8, co=8
                            ),
                        )
                    dwpw = dwpp.tile([128, NTAP, 128], BF16, tag=f"dwpw{b}{t}")
                    dwpws[b, t] = dwpw
                    for gx, (g0, ng) in enumerate(GRPS):
                        psd = psum.tile([128, 512], F32, tag="mm")
                        for gi in range(ng):
                            g = g0 + gi
                            gh, gl = g // 8, g % 8
                            nc.tensor.matmul(
                                psd[:, gi * 80 : (gi + 1) * 80].rearrange(
                                    "p (tap co) -> p tap co", tap=NTAP
                                ),
                                idT[:, g, :],
                                dsrcc[:, gh, gl, :].rearrange(
                                    "i (co tap) -> i tap co", co=8
                                ),
                                start=True,
                                stop=True,
                            )
                        dst = dwpw[:, :, 8 * g0 : 8 * (g0 + ng)].rearrange(
                            "p tap (g co) -> p g tap co", g=ng
                        )
                        src = psd[:, : ng * 80].rearrange(
                            "p (g tap co) -> p g tap co", g=ng, tap=NTAP
                        )
                        if gx % 2:
                            nc.vector.tensor_copy(dst, src)
                        else:
                            nc.scalar.mul(dst, src, 1.0)
                xp = xps[b]
                yp1 = yp1s[b]
                for t in range(NT):
                    dwpw = dwpws[b, t]
                    ysb = actp.tile([128, HW], BF16, tag="ysb")
                    for hh in range(2):
                        ps = psum.tile([128, 512], F32, tag="mm")
                        for kdi in range(3):
                            for kdj in range(3):
                                pos = kdi * 3 + kdj
                                nc.tensor.matmul(
                                    ps[:],
                                    dwpw[:, pos, :],
                                    xp[:, t, kdi + 16 * hh : kdi + 16 * hh + 16, kdj : kdj + 32],
                                    start=(pos == 0),
                                    stop=(pos == 8),
                                )
                        nc.vector.tensor_copy(ysb[:, 512 * hh : 512 * (hh + 1)], ps[:])
                    for hh in range(2):
                        ps2 = psum.tile([128, 512], F32, tag="mm")
                        nc.tensor.matmul(
                            ps2[:],
                            dwpw[:, 9, :],
                            ysb[:, 512 * hh : 512 * (hh + 1)],
                            start=True,
                            stop=True,
                        )
                        nc.scalar.activation(
                            out=yp1[:, t, 1 + 16 * hh : 17 + 16 * hh, 1:33],
                            in_=ps2[:].rearrange("p (h w) -> p h w", h=16),
                            func=AF.Identity,
                            bias=biasc[:, t, b : b + 1],
                            scale=1.0,
                        )

            # ---------------- phase B: conv1 (512 -> 512) + relu ----------------
            # yp2 buffers reuse the xp slots (free once dynconv has read them)
            yp2s = []
            for b in range(BPC):
                yp2 = pad3.tile([128, NT, 34, 34], BF16, tag="padbuf")
                yp2s.append(yp2)
                _zero_border(yp2)
            for m in range(NT):
                pss2 = [
                    psum.tile([128, 512], F32, tag="mm", name=f"pb{i}")
                    for i in range(2 * BPC)
                ]
                for k in range(NT):
                    for b in range(BPC):
                        for hh in range(2):
                            ps = pss2[2 * b + hh]
                            for kdi in range(3):
                                for kdj in range(3):
                                    pos = kdi * 3 + kdj
                                    nc.tensor.matmul(
                                        ps[:],
                                        w1sb[k][:, pos, 128 * m : 128 * (m + 1)],
                                        yp1s[b][:, k, kdi + 16 * hh : kdi + 16 * hh + 16, kdj : kdj + 32],
                                        start=(k == 0 and pos == 0),
                                        stop=(k == NT - 1 and pos == 8),
                                    )
                for b in range(BPC):
                    for hh in range(2):
                        nc.scalar.activation(
                            out=yp2s[b][:, m, 1 + 16 * hh : 17 + 16 * hh, 1:33],
                            in_=pss2[2 * b + hh][:].rearrange("p (h w) -> p h w", h=16),
                            func=AF.Relu,
                            bias=b1sb[:, m : m + 1],
                            scale=1.0,
                        )

            # ------- phase C: conv2 (512 -> 256) + relu + 2x upsample -------
            # h-half split: each half's upsample-expand and output DMA
            # pipeline under the other half's matmuls, shortening the tail
            for m2 in range(NM2):
                for b in range(BPC):
                    for hh in range(2):
                        for hb in range(2):
                            ous = outp.tile([128, 8, 2, 64], F32, tag="ous")
                            ps = psum.tile([128, 512], F32, tag="mm")
                            r0 = 16 * hh + 8 * hb
                            for k in range(NT):
                                for kdi in range(3):
                                    for kdj in range(3):
                                        pos = kdi * 3 + kdj
                                        nc.tensor.matmul(
                                            ps[:, :256],
                                            w2sb[k][:, pos, 128 * m2 : 128 * (m2 + 1)],
                                            yp2s[b][:, k, kdi + r0 : kdi + r0 + 8, kdj : kdj + 32],
                                            start=(k == 0 and pos == 0),
                                            stop=(k == NT - 1 and pos == 8),
                                        )
                            for two in range(2):
                                for dup in range(2):
                                    ov = ous[:, :, two, :].rearrange(
                                        "p h (w dup) -> p h dup w", dup=2
                                    )[:, :, dup, :]
                                    iv = ps[:, :256].rearrange("p (h w) -> p h w", h=8)
                                    if two:
                                        nc.vector.tensor_scalar(
                                            out=ov,
                                            in0=iv,
                                            scalar1=b2sb[:, m2 : m2 + 1],
                                            scalar2=0.0,
                                            op0=AluOpType.add,
                                            op1=AluOpType.max,
                                        )
                                    else:
                                        nc.scalar.activation(
                                            out=ov,
                                            in_=iv,
                                            func=AF.Relu,
                                            bias=b2sb[:, m2 : m2 + 1],
                                            scale=1.0,
                                        )
                            nc.sync.dma_start(
                                out=yout[b, 128 * m2 : 128 * (m2 + 1), :, :]
                                .rearrange("c (q h2) w -> c q h2 w", q=4)[:, 2 * hh + hb]
                                .rearrange("c (h two) w -> c h two w", two=2),
                                in_=ous[:],
                            )

    nc.compile()
    return nc


def _repack(inputs):
    bf = ml_dtypes.bfloat16
    kp_sw = np.ascontiguousarray(inputs["kp_sw"], dtype=np.float32)
    kp_sb = np.ascontiguousarray(inputs["kp_sb"], dtype=np.float32)
    kp_pw = np.ascontiguousarray(inputs["kp_pw"], dtype=np.float32)
    kp_pb = np.ascontiguousarray(inputs["kp_pb"], dtype=np.float32)
    kp_bw = np.ascontiguousarray(inputs["kp_bw"], dtype=np.float32)
    kp_bb = np.ascontiguousarray(inputs["kp_bb"], dtype=np.float32)
    dec_w1 = np.ascontiguousarray(inputs["dec_w1"], dtype=np.float32)
    dec_b1 = np.ascontiguousarray(inputs["dec_b1"], dtype=np.float32)
    dec_w2 = np.ascontiguousarray(inputs["dec_w2"], dtype=np.float32)
    dec_b2 = np.ascontiguousarray(inputs["dec_b2"], dtype=np.float32)

    # column permutation: position (t, g, i, co) <- original o = (c_out, i)
    O = np.arange(C * CG).reshape(NT, 16, CG, CG)  # (t, g, co, i), o-major
    P = O.transpose(0, 1, 3, 2).reshape(-1)        # (t, g, i, co)

    kpsw = np.empty((9 * S + 1, C * CG), dtype=np.float32)
    kpsw[: 9 * S] = (
        kp_sw[P].reshape(C * CG, S, 3, 3).transpose(2, 3, 1, 0).reshape(9 * S, C * CG)
    )  # rows in k-order (di, dj, s)
    kpsw[9 * S] = kp_sb[P]

    kppw = np.empty((S + 1, C * CG), dtype=np.float32)
    kppw[:S] = kp_pw[P].T
    kppw[S] = kp_pb[P]

    kpbw = np.empty((S + 1, C), dtype=np.float32)
    kpbw[:S] = kp_bw.T
    kpbw[S] = kp_bb

    w1 = np.ascontiguousarray(dec_w1.transpose(2, 3, 1, 0).reshape(9, C, C))
    w2 = np.ascontiguousarray(dec_w2.transpose(2, 3, 1, 0).reshape(9, C, CO))

    wq = np.ascontiguousarray(inputs["w"], dtype=np.float32).reshape(B, 4, 128, 9)
    shared = {
        "kpsws": kpsw.astype(bf),
        "kppws": kppw.astype(bf),
        "w1t": w1.astype(bf),
        "w2t": w2.astype(bf),
        "b1d": dec_b1,
        "b2d": dec_b2,
        "kpbw": kpbw.astype(bf),
    }
    slices = []
    for c in range(NCORES):
        slices.append(
            {
                "wownr": np.ascontiguousarray(
                    wq[BPC * c : BPC * (c + 1)].transpose(2, 1, 0, 3)
                ),
            }
        )
    return shared, slices


def kernel(**inputs):
    if "nc" not in _CACHE:
        _CACHE["nc"] = _build()
    nc = _CACHE["nc"]

    shared, slices = _repack(inputs)
    x = np.ascontiguousarray(inputs["x"], dtype=np.float32).astype(
        __import__("ml_dtypes").bfloat16
    )

    in_maps = []
    for c in range(NCORES):
        sl = slice(BPC * c, BPC * (c + 1))
        in_maps.append({"x2": x[sl], **shared, **slices[c]})

    res = run_bass_kernel_spmd(nc, in_maps, list(range(NCORES))).results
    return np.concatenate([r["yout"] for r in res], axis=0)


# revision 10
# speedup vs baseline: 1.1190x; 1.1190x over previous
"""Trainium2 Bass kernel for nn_DecoderBlock (dynamic-conv decoder block).

v3: NO collective. Pure data-parallel over batch (2 samples/core); the
kernel-predictor weights (kpsw [4609, 4096] bf16, ~38 MB) are replicated
and streamed once per core, and each core computes all 4096 predictor
columns for only its OWN 2 samples. This removes the AllToAll that
previously serialized core 0 against the staggered per-core NEFF
launches (~9 ms apart -> ~70 ms stall on the profiled core).

Math per sample (C=512, G=64, cg=8, H=W=32, S=512, Cout=256):
  dw   = conv3x3(reflect_pad(w), kp_sw) + kp_sb        # kernel predictor
  pw   = pooled @ kp_pw.T + kp_pb ;  bias = pooled @ kp_bw.T + kp_bb
  xn   = instance_norm(x)
  y    = grouped_dynconv3x3(reflect_pad(xn), dw)       # per-sample weights
  y    = grouped_pointwise(pw, y) + bias
  y    = relu(conv3x3(y, dec_w1) + b1)
  y    = relu(conv3x3(y, dec_w2) + b2)
  out  = nearest_upsample_2x(y)

The predictor matmuls accumulate [128-col-chunk, (b, opos)] regions in two
PSUM banks over the (9 kpos x 4 s-chunk) contraction; results land in
dwTs [128, 32, b, tap] and are DMA'd to a DRAM scratch in the same slab
layout the old AllToAll produced, so the dynamic-conv phase (block-diag
[128,128] weight tiles built with identity-selector matmuls) is unchanged.

Queue plan: kpsw/kppw stream alternates sync/gpsimd; x loads on the DVE
queue; small predictor/bias weights + the identity build on ACT's queue;
w1t on gpsimd and w2t on ACT (needed only by phases B/C, so they drain
after the predictor stream).
"""

import sys

sys.path.insert(0, "/opt/trn_rl_repo")

import numpy as np
import ml_dtypes

import concourse.bacc as bacc
import concourse.tile as tile
from concourse import mybir
from concourse.alu_op_type import AluOpType
from concourse.bass_utils import run_bass_kernel_spmd

F32 = mybir.dt.float32
BF16 = mybir.dt.bfloat16
AF = mybir.ActivationFunctionType

NCORES = 8
B = 16           # total batch
BPC = 2          # samples per core
C = 512          # in channels
CO = 256         # out channels
S = 512          # style dim
G = 64           # groups
CG = 8           # channels per group
H = W = 32
HW = H * W
NT = C // 128    # 4 channel tiles
NM2 = CO // 128  # 2 out-channel tiles
EPS = 1e-5
SCR = 128 * 129  # 16512; span of the diag-embedded identity build
NCOL = C * CG    # 4096 predictor columns (all on every core now)
NCC = NCOL // 128  # 32 column chunks
NTAP = 10        # 9 dw taps + the pw tap
CCW = NTAP * 512  # 5120 = per-slab width in the dw scratch buffer
BPO = BPC * 9    # 18 = (own sample, out-position) columns in predictor matmul

_CACHE = {}


def _build():
    nc = bacc.Bacc(None, target_bir_lowering=False)

    x2 = nc.declare_dram_parameter("x2", [BPC, C, H, W], BF16, isOutput=False)
    wownr = nc.declare_dram_parameter("wownr", [128, 4, BPC, 9], F32, isOutput=False)
    kpsws = nc.declare_dram_parameter("kpsws", [9 * S + 1, NCOL], BF16, isOutput=False)
    kppws = nc.declare_dram_parameter("kppws", [S + 1, NCOL], BF16, isOutput=False)
    kpbw = nc.declare_dram_parameter("kpbw", [S + 1, C], BF16, isOutput=False)
    w1t = nc.declare_dram_parameter("w1t", [9, C, C], BF16, isOutput=False)
    w2t = nc.declare_dram_parameter("w2t", [9, C, CO], BF16, isOutput=False)
    b1d = nc.declare_dram_parameter("b1d", [C], F32, isOutput=False)
    b2d = nc.declare_dram_parameter("b2d", [CO], F32, isOutput=False)
    yout = nc.declare_dram_parameter("yout", [BPC, CO, 2 * H, 2 * W], F32, isOutput=True)

    ccout = nc.dram_tensor("ccout", [B, CCW], BF16)
    idscr = nc.dram_tensor("idscr", [SCR], BF16)

    with tile.TileContext(nc) as tc:
        with (
            tc.tile_pool(name="consts", bufs=1) as consts,
            tc.tile_pool(name="kstream", bufs=4) as kstream,
            tc.tile_pool(name="wres", bufs=1) as wres,
            tc.tile_pool(name="dwp", bufs=1) as dwpp,
            tc.tile_pool(name="blk", bufs=4) as blkp,
            tc.tile_pool(name="act", bufs=3) as actp,
            tc.tile_pool(name="pad3", bufs=4) as pad3,
            tc.tile_pool(name="outp", bufs=4) as outp,
            tc.tile_pool(name="psum", bufs=6, space="PSUM") as psum,
            tc.tile_pool(name="psumP", bufs=2, space="PSUM") as psumP,
        ):
            # ---------------- persistent small constants ----------------
            epsb = consts.tile([128, 1], F32, tag="epsb")
            nc.vector.memset(epsb[:], EPS)
            # prime the ACT function table while the first DMAs run
            actwarm = consts.tile([128, 1], F32, tag="actwarm")
            nc.scalar.mul(actwarm[:], epsb[:], 1.0)
            onesf = consts.tile([1, BPO], F32, tag="onesf")
            nc.vector.memset(onesf[:], 1.0)
            ones = consts.tile([1, BPO], BF16, tag="ones")
            nc.vector.tensor_copy(ones[:], onesf[:])
            zb16 = consts.tile([128, 132], BF16, tag="zb16")
            nc.vector.memset(zb16[:], 0.0)
            onescol = consts.tile([128, 1], BF16, tag="onescol")
            nc.vector.memset(onescol[:], 1.0)

            # ---------------- own style maps + pooled ----------------
            wosb = consts.tile([128, 4, BPC, 9], F32, tag="wosb")
            nc.sync.dma_start(out=wosb[:], in_=wownr[:, :, :, :])
            # bias-row and small-weight loads, all on queues that the big
            # kpsw stream does not use
            rb = consts.tile([1, NCOL], BF16, tag="rb")
            nc.scalar.dma_start(out=rb[:], in_=kpsws[9 * S : 9 * S + 1, :])
            rbp = consts.tile([1, NCOL], BF16, tag="rbp")
            nc.scalar.dma_start(out=rbp[:], in_=kppws[S : S + 1, :])
            b1sb = consts.tile([128, NT], F32, tag="b1sb")
            nc.scalar.dma_start(out=b1sb[:, :], in_=b1d.rearrange("(m c) -> c m", c=128))
            b2sb = consts.tile([128, NM2], F32, tag="b2sb")
            nc.scalar.dma_start(out=b2sb[:, :], in_=b2d.rearrange("(m c) -> c m", c=128))
            kbsb = consts.tile([128, 4, C], BF16, tag="kbsb")
            nc.scalar.dma_start(
                out=kbsb[:], in_=kpbw[:S, :].rearrange("(q s) c -> s q c", q=4)
            )
            rbb = consts.tile([1, C], BF16, tag="rbb")
            nc.scalar.dma_start(out=rbb[:], in_=kpbw[S : S + 1, :])

            # identity for the block-diag build: idT[i, g, m] = (m == 8g+i),
            # via a DRAM stride-129 diagonal embedding
            nc.scalar.dma_start(
                out=idscr.rearrange("(p c) -> p c", c=129), in_=zb16[:, :129]
            )
            nc.scalar.dma_start(
                out=idscr.rearrange("(p c) -> p c", c=129)[:, :1], in_=onescol[:]
            )
            idT = consts.tile([8, 16, 128], BF16, tag="idT")
            nc.scalar.dma_start(
                out=idT[:],
                in_=idscr[: 128 * 128]
                .rearrange("(gi m) -> gi m", m=128)
                .rearrange("(g i) m -> i g m", g=16),
            )

            # reflect-pad 3x3 -> 5x5 (batched over b), bf16; on ACT to keep
            # DVE free for the im2col below
            wp = consts.tile([128, 4, BPC, 5, 5], BF16, tag="wp")
            for q in range(4):
                w3 = wosb[:, q, :, :].rearrange("p b (kh kw) -> p b kh kw", kh=3)
                if q < 2:
                    nc.scalar.mul(wp[:, q, :, 1:4, 1:4], w3, 1.0)
                    nc.scalar.mul(wp[:, q, :, 1:4, 0:1], w3[:, :, :, 1:2], 1.0)
                    nc.scalar.mul(wp[:, q, :, 1:4, 4:5], w3[:, :, :, 1:2], 1.0)
                    nc.scalar.mul(wp[:, q, :, 0, :], wp[:, q, :, 2, :], 1.0)
                    nc.scalar.mul(wp[:, q, :, 4, :], wp[:, q, :, 2, :], 1.0)
                else:
                    nc.vector.tensor_copy(wp[:, q, :, 1:4, 1:4], w3)
                    nc.vector.tensor_copy(wp[:, q, :, 1:4, 0:1], w3[:, :, :, 1:2])
                    nc.vector.tensor_copy(wp[:, q, :, 1:4, 4:5], w3[:, :, :, 1:2])
                    nc.vector.tensor_copy(wp[:, q, :, 0, :], wp[:, q, :, 2, :])
                    nc.vector.tensor_copy(wp[:, q, :, 4, :], wp[:, q, :, 2, :])

            # im2col of padded style map: [s-chunk, kpos, q, (b, opos)]
            xw = consts.tile([128, 9, 4, BPO], BF16, tag="xw")
            for di in range(3):
                for dj in range(3):
                    nc.vector.tensor_copy(
                        xw[:, di * 3 + dj, :, :].rearrange(
                            "p q (b i j) -> p q b i j", b=BPC, i=3
                        ),
                        wp[:, :, :, di : di + 3, dj : dj + 3],
                    )

            pooledo_f = consts.tile([128, 4, BPC], F32, tag="pooledof")
            for q in range(4):
                nc.vector.tensor_reduce(
                    out=pooledo_f[:, q, :],
                    in_=wosb[:, q, :, :],
                    axis=mybir.AxisListType.X,
                    op=AluOpType.add,
                )
            pooledo = consts.tile([128, 4, BPC], BF16, tag="pooledo")
            nc.scalar.mul(pooledo[:, :, :], pooledo_f[:, :, :], 1.0 / 9.0)

            # instance norm -> reflect-padded xn (bf16); x loads ride the
            # DVE queue so they overlap the predictor weight stream
            xps = []
            for b in range(BPC):
                xp = pad3.tile([128, NT, 34, 34], BF16, tag="padbuf")
                xps.append(xp)
                for t in range(NT):
                    xsb = actp.tile([128, HW], BF16, tag="xsb")
                    nc.scalar.dma_start(
                        out=xsb[:],
                        in_=x2[b, 128 * t : 128 * (t + 1), :, :].rearrange(
                            "c h w -> c (h w)"
                        ),
                    )
                    st = actp.tile([128, 2, 6], F32, tag="bnst")
                    xsb2 = xsb[:].rearrange("p (s f) -> p s f", f=512)
                    for sg in range(2):
                        nc.vector.bn_stats(out=st[:, sg, :], in_=xsb2[:, sg, :])
                    mv = actp.tile([128, 2], F32, tag="bnmv")
                    nc.vector.bn_aggr(out=mv[:], in_=st[:])
                    rstd = actp.tile([128, 1], F32, tag="rstd")
                    nc.scalar.activation(
                        out=rstd[:], in_=mv[:, 1:2], func=AF.Sqrt, bias=epsb[:], scale=1.0
                    )
                    nc.vector.reciprocal(out=rstd[:], in_=rstd[:])
                    nc.vector.tensor_scalar(
                        out=xp[:, t, 1:33, 1:33],
                        in0=xsb[:].rearrange("p (h w) -> p h w", h=H),
                        scalar1=mv[:, 0:1],
                        scalar2=rstd[:],
                        op0=AluOpType.subtract,
                        op1=AluOpType.mult,
                    )
                    nc.vector.tensor_copy(xp[:, t, 1:33, 0:1], xp[:, t, 1:33, 2:3])
                    nc.vector.tensor_copy(xp[:, t, 1:33, 33:34], xp[:, t, 1:33, 31:32])
                    nc.vector.tensor_copy(xp[:, t, 0, :], xp[:, t, 2, :])
                    nc.vector.tensor_copy(xp[:, t, 33, :], xp[:, t, 31, :])

            # ------- predictor: dw cols (ALL 32 chunks, own samples) -------
            # psum regions: pd0 holds chunks 0..27 at [18cc, 18cc+18);
            # pd1 holds chunks 28..31 then the 32 pw 2-wide regions at 72+.
            pd0 = psumP.tile([128, 512], F32, tag="pd", name="pd0")
            pd1 = psumP.tile([128, 512], F32, tag="pd", name="pd1")

            def _dwreg(cc):
                if cc < 28:
                    return pd0[:, 18 * cc : 18 * cc + 18]
                return pd1[:, 18 * (cc - 28) : 18 * (cc - 28) + 18]

            def _pwreg(cc):
                return pd1[:, 72 + 2 * cc : 74 + 2 * cc]

            # ------- bias predictor (own 2 samples); PE is idle here -------
            biasc = consts.tile([128, NT, BPC], F32, tag="biasc")
            for m in range(NT):
                ps3 = psum.tile([128, 512], F32, tag="mm")
                for q in range(4):
                    nc.tensor.matmul(
                        ps3[:, :BPC],
                        kbsb[:, q, 128 * m : 128 * (m + 1)],
                        pooledo[:, q, :],
                        start=(q == 0),
                        stop=False,
                    )
                nc.tensor.matmul(
                    ps3[:, :BPC],
                    rbb[:1, 128 * m : 128 * (m + 1)],
                    ones[:1, :BPC],
                    start=False,
                    stop=True,
                )
                nc.vector.tensor_copy(biasc[:, m, :], ps3[:, :BPC])

            # zero-padded output buffers for conv1, prepped before the
            # stream so DVE is free during the pipelined phase A
            def _zero_border(yp):
                for t in range(NT):
                    nc.vector.tensor_copy(yp[:, t, 0, :], zb16[:, :34])
                    nc.vector.tensor_copy(yp[:, t, 33, :], zb16[:, :34])
                    nc.vector.tensor_copy(
                        yp[:, t, 1:33, 0:1],
                        zb16[:, :32].rearrange("p (a c) -> p a c", c=1),
                    )
                    nc.vector.tensor_copy(
                        yp[:, t, 1:33, 33:34],
                        zb16[:, :32].rearrange("p (a c) -> p a c", c=1),
                    )

            yp1s = []
            for b in range(BPC):
                yp1 = pad3.tile([128, NT, 34, 34], BF16, tag="padbuf")
                yp1s.append(yp1)
                _zero_border(yp1)

            # ---- predictor stream + pipelined phase A ----
            # ch-outer streaming: column group ch = (t, gh) = 4 chunks of 128
            # cols; its 9 kpsw chunks + 1 kppw chunk arrive, its chains close,
            # dwTs copies + ccout slab writes for (t, gh) fire, and after the
            # odd ch of each pair the whole t-tile's dynamic conv runs while
            # later groups stream. PSUM "start" marks the whole 2 KB bank
            # zero-on-next-write, so each bank gets exactly ONE start (its
            # first matmul: pd0 = dw cc0, pd1 = pw cc0) and ONE stop (pd0:
            # dw-bias cc27, pd1: pw-bias cc31).
            dwTs = consts.tile([128, NCC, BPC, NTAP], BF16, tag="dwTs")
            GRPS = ((0, 6), (6, 6), (12, 4))  # psum passes over the 16 groups

            def _phase_a(b, t):
                xp = xps[b]
                yp1 = yp1s[b]
                dsrcc = blkp.tile([8, 2, 8, 8 * NTAP], BF16, tag="dsrcc")
                for gh in range(2):
                    nc.sync.dma_start(
                        out=dsrcc[:, gh, :, :],
                        in_=ccout[2 * (2 * t + gh) + b, :].rearrange(
                            "(gl i co tap) -> i gl (co tap)", gl=8, i=8, co=8
                        ),
                    )
                dwpw = dwpp.tile([128, NTAP, 128], BF16, tag=f"dwpw{b % 2}{t % 2}")
                for gx, (g0, ng) in enumerate(GRPS):
                    psd = psum.tile([128, 512], F32, tag="mm")
                    for gi in range(ng):
                        g = g0 + gi
                        gh, gl = g // 8, g % 8
                        nc.tensor.matmul(
                            psd[:, gi * 80 : (gi + 1) * 80].rearrange(
                                "p (tap co) -> p tap co", tap=NTAP
                            ),
                            idT[:, g, :],
                            dsrcc[:, gh, gl, :].rearrange(
                                "i (co tap) -> i tap co", co=8
                            ),
                            start=True,
                            stop=True,
                        )
                    dst = dwpw[:, :, 8 * g0 : 8 * (g0 + ng)].rearrange(
                        "p tap (g co) -> p g tap co", g=ng
                    )
                    src = psd[:, : ng * 80].rearrange(
                        "p (g tap co) -> p g tap co", g=ng, tap=NTAP
                    )
                    if gx % 2:
                        nc.vector.tensor_copy(dst, src)
                    else:
                        nc.scalar.mul(dst, src, 1.0)
                ysb = actp.tile([128, HW], BF16, tag="ysb")
                for hh in range(2):
                    ps = psum.tile([128, 512], F32, tag="mm")
                    for kdi in range(3):
                        for kdj in range(3):
                            pos = kdi * 3 + kdj
                            nc.tensor.matmul(
                                ps[:],
                                dwpw[:, pos, :],
                                xp[:, t, kdi + 16 * hh : kdi + 16 * hh + 16, kdj : kdj + 32],
                                start=(pos == 0),
                                stop=(pos == 8),
                            )
                    nc.vector.tensor_copy(ysb[:, 512 * hh : 512 * (hh + 1)], ps[:])
                for hh in range(2):
                    ps2 = psum.tile([128, 512], F32, tag="mm")
                    nc.tensor.matmul(
                        ps2[:],
                        dwpw[:, 9, :],
                        ysb[:, 512 * hh : 512 * (hh + 1)],
                        start=True,
                        stop=True,
                    )
                    nc.scalar.activation(
                        out=yp1[:, t, 1 + 16 * hh : 17 + 16 * hh, 1:33],
                        in_=ps2[:].rearrange("p (h w) -> p h w", h=16),
                        func=AF.Identity,
                        bias=biasc[:, t, b : b + 1],
                        scale=1.0,
                    )

            for ch in range(8):
                for kpos in range(9):
                    ksb = kstream.tile([128, 4, 512], BF16, tag="ksb")
                    dma = nc.sync.dma_start if (kpos + ch) % 2 else nc.gpsimd.dma_start
                    dma(
                        out=ksb[:],
                        in_=kpsws[
                            kpos * 512 : (kpos + 1) * 512, 512 * ch : 512 * (ch + 1)
                        ].rearrange("(q s) c -> s q c", q=4),
                    )
                    for q in range(4):
                        for sub in range(4):
                            cc = 4 * ch + sub
                            nc.tensor.matmul(
                                _dwreg(cc),
                                ksb[:, q, 128 * sub : 128 * (sub + 1)],
                                xw[:, kpos, q, :],
                                start=(cc == 0 and kpos == 0 and q == 0),
                                stop=False,
                            )
                # pw tap for this column group
                kpb_t = kstream.tile([128, 4, 512], BF16, tag="ksb")
                dma = nc.sync.dma_start if ch % 2 else nc.gpsimd.dma_start
                dma(
                    out=kpb_t[:],
                    in_=kppws[:S, 512 * ch : 512 * (ch + 1)].rearrange(
                        "(q s) c -> s q c", q=4
                    ),
                )
                for q in range(4):
                    for sub in range(4):
                        cc = 4 * ch + sub
                        nc.tensor.matmul(
                            _pwreg(cc),
                            kpb_t[:, q, 128 * sub : 128 * (sub + 1)],
                            pooledo[:, q, :],
                            start=(ch == 0 and q == 0 and sub == 0),
                            stop=False,
                        )
                # close this group's chains and stage its dwTs columns
                for sub in range(4):
                    cc = 4 * ch + sub
                    nc.tensor.matmul(
                        _dwreg(cc),
                        rb[:1, 128 * cc : 128 * (cc + 1)],
                        ones[:1, :BPO],
                        start=False,
                        stop=(cc == 27),
                    )
                    nc.tensor.matmul(
                        _pwreg(cc),
                        rbp[:1, 128 * cc : 128 * (cc + 1)],
                        ones[:1, :BPC],
                        start=False,
                        stop=(cc == 31),
                    )
                for sub in range(4):
                    cc = 4 * ch + sub
                    if cc % 2:
                        nc.scalar.mul(
                            dwTs[:, cc, :, :9],
                            _dwreg(cc).rearrange("p (b pos) -> p b pos", b=BPC),
                            1.0,
                        )
                        nc.scalar.mul(dwTs[:, cc, :, 9], _pwreg(cc), 1.0)
                    else:
                        nc.vector.tensor_copy(
                            dwTs[:, cc, :, :9],
                            _dwreg(cc).rearrange("p (b pos) -> p b pos", b=BPC),
                        )
                        nc.vector.tensor_copy(dwTs[:, cc, :, 9], _pwreg(cc))
                # slab write for (t, gh) = (ch // 2, ch % 2): within-slab
                # index = (128*blk + p)*10 + tap, cc = 8t + 4gh + blk
                t, gh = ch // 2, ch % 2
                for b in range(BPC):
                    nc.sync.dma_start(
                        out=ccout.rearrange(
                            "(s2 b) (blk p pos) -> s2 b p blk pos",
                            b=BPC, blk=4, pos=NTAP,
                        )[2 * t + gh, b],
                        in_=dwTs[:, 8 * t + 4 * gh : 8 * t + 4 * gh + 4, b, :],
                    )
                # after the odd half, the whole t-tile's dw is in DRAM:
                # run its dynamic conv for both samples under the stream
                if gh == 1:
                    for b in range(BPC):
                        _phase_a(b, t)

            # decoder conv weights, SBUF-resident for the rest of the kernel.
            # Both ride the gpsimd queue BEHIND the predictor stream, so their
            # transfers start only once the stream has drained.
            w1sb = []
            for k in range(NT):
                t_ = wres.tile([128, 9, C], BF16, tag=f"w1k{k}")
                w1sb.append(t_)
                for p3 in range(3):
                    nc.gpsimd.dma_start(
                        out=t_[:, 3 * p3 : 3 * (p3 + 1), :],
                        in_=w1t[3 * p3 : 3 * (p3 + 1), 128 * k : 128 * (k + 1), :]
                        .rearrange("pos p co -> p pos co"),
                    )
            w2sb = []
            for k in range(NT):
                t_ = wres.tile([128, 9, CO], BF16, tag=f"w2k{k}")
                w2sb.append(t_)
                nc.gpsimd.dma_start(
                    out=t_[:],
                    in_=w2t[:, 128 * k : 128 * (k + 1), :].rearrange(
                        "pos p co -> p pos co"
                    ),
                )

            # ---------------- phase B: conv1 (512 -> 512) + relu ----------------
            # yp2 buffers reuse the xp slots (free once dynconv has read them)
            yp2s = []
            for b in range(BPC):
                yp2 = pad3.tile([128, NT, 34, 34], BF16, tag="padbuf")
                yp2s.append(yp2)
                _zero_border(yp2)
            for m in range(NT):
                pss2 = [
                    psum.tile([128, 512], F32, tag="mm", name=f"pb{i}")
                    for i in range(2 * BPC)
                ]
                for k in range(NT):
                    for b in range(BPC):
                        for hh in range(2):
                            ps = pss2[2 * b + hh]
                            for kdi in range(3):
                                for kdj in range(3):
                                    pos = kdi * 3 + kdj
                                    nc.tensor.matmul(
                                        ps[:],
                                        w1sb[k][:, pos, 128 * m : 128 * (m + 1)],
                                        yp1s[b][:, k, kdi + 16 * hh : kdi + 16 * hh + 16, kdj : kdj + 32],
                                        start=(k == 0 and pos == 0),
                                        stop=(k == NT - 1 and pos == 8),
                                    )
                for b in range(BPC):
                    for hh in range(2):
                        nc.scalar.activation(
                            out=yp2s[b][:, m, 1 + 16 * hh : 17 + 16 * hh, 1:33],
                            in_=pss2[2 * b + hh][:].rearrange("p (h w) -> p h w", h=16),
                            func=AF.Relu,
                            bias=b1sb[:, m : m + 1],
                            scale=1.0,
                        )

            # ------- phase C: conv2 (512 -> 256) + relu + 2x upsample -------
            # h-half split: each half's upsample-expand and output DMA
            # pipeline under the other half's matmuls, shortening the tail
            for m2 in range(NM2):
                for b in range(BPC):
                    for hh in range(2):
                        for hb in range(2):
                            ous = outp.tile([128, 8, 2, 64], F32, tag="ous")
                            ps = psum.tile([128, 512], F32, tag="mm")
                            r0 = 16 * hh + 8 * hb
                            for k in range(NT):
                                for kdi in range(3):
                                    for kdj in range(3):
                                        pos = kdi * 3 + kdj
                                        nc.tensor.matmul(
                                            ps[:, :256],
                                            w2sb[k][:, pos, 128 * m2 : 128 * (m2 + 1)],
                                            yp2s[b][:, k, kdi + r0 : kdi + r0 + 8, kdj : kdj + 32],
                                            start=(k == 0 and pos == 0),
                                            stop=(k == NT - 1 and pos == 8),
                                        )
                            for two in range(2):
                                for dup in range(2):
                                    ov = ous[:, :, two, :].rearrange(
                                        "p h (w dup) -> p h dup w", dup=2
                                    )[:, :, dup, :]
                                    iv = ps[:, :256].rearrange("p (h w) -> p h w", h=8)
                                    if two:
                                        nc.vector.tensor_scalar(
                                            out=ov,
                                            in0=iv,
                                            scalar1=b2sb[:, m2 : m2 + 1],
                                            scalar2=0.0,
                                            op0=AluOpType.add,
                                            op1=AluOpType.max,
                                        )
                                    else:
                                        nc.scalar.activation(
                                            out=ov,
                                            in_=iv,
                                            func=AF.Relu,
                                            bias=b2sb[:, m2 : m2 + 1],
                                            scale=1.0,
                                        )
                            nc.sync.dma_start(
                                out=yout[b, 128 * m2 : 128 * (m2 + 1), :, :]
                                .rearrange("c (q h2) w -> c q h2 w", q=4)[:, 2 * hh + hb]
                                .rearrange("c (h two) w -> c h two w", two=2),
                                in_=ous[:],
                            )

    nc.compile()
    return nc


def _repack(inputs):
    bf = ml_dtypes.bfloat16
    kp_sw = np.ascontiguousarray(inputs["kp_sw"], dtype=np.float32)
    kp_sb = np.ascontiguousarray(inputs["kp_sb"], dtype=np.float32)
    kp_pw = np.ascontiguousarray(inputs["kp_pw"], dtype=np.float32)
    kp_pb = np.ascontiguousarray(inputs["kp_pb"], dtype=np.float32)
    kp_bw = np.ascontiguousarray(inputs["kp_bw"], dtype=np.float32)
    kp_bb = np.ascontiguousarray(inputs["kp_bb"], dtype=np.float32)
    dec_w1 = np.ascontiguousarray(inputs["dec_w1"], dtype=np.float32)
    dec_b1 = np.ascontiguousarray(inputs["dec_b1"], dtype=np.float32)
    dec_w2 = np.ascontiguousarray(inputs["dec_w2"], dtype=np.float32)
    dec_b2 = np.ascontiguousarray(inputs["dec_b2"], dtype=np.float32)

    # column permutation: position (t, g, i, co) <- original o = (c_out, i)
    O = np.arange(C * CG).reshape(NT, 16, CG, CG)  # (t, g, co, i), o-major
    P = O.transpose(0, 1, 3, 2).reshape(-1)        # (t, g, i, co)

    kpsw = np.empty((9 * S + 1, C * CG), dtype=np.float32)
    kpsw[: 9 * S] = (
        kp_sw[P].reshape(C * CG, S, 3, 3).transpose(2, 3, 1, 0).reshape(9 * S, C * CG)
    )  # rows in k-order (di, dj, s)
    kpsw[9 * S] = kp_sb[P]

    kppw = np.empty((S + 1, C * CG), dtype=np.float32)
    kppw[:S] = kp_pw[P].T
    kppw[S] = kp_pb[P]

    kpbw = np.empty((S + 1, C), dtype=np.float32)
    kpbw[:S] = kp_bw.T
    kpbw[S] = kp_bb

    w1 = np.ascontiguousarray(dec_w1.transpose(2, 3, 1, 0).reshape(9, C, C))
    w2 = np.ascontiguousarray(dec_w2.transpose(2, 3, 1, 0).reshape(9, C, CO))

    wq = np.ascontiguousarray(inputs["w"], dtype=np.float32).reshape(B, 4, 128, 9)
    shared = {
        "kpsws": kpsw.astype(bf),
        "kppws": kppw.astype(bf),
        "w1t": w1.astype(bf),
        "w2t": w2.astype(bf),
        "b1d": dec_b1,
        "b2d": dec_b2,
        "kpbw": kpbw.astype(bf),
    }
    slices = []
    for c in range(NCORES):
        slices.append(
            {
                "wownr": np.ascontiguousarray(
                    wq[BPC * c : BPC * (c + 1)].transpose(2, 1, 0, 3)
                ),
            }
        )
    return shared, slices


def kernel(**inputs):
    if "nc" not in _CACHE:
        _CACHE["nc"] = _build()
    nc = _CACHE["nc"]

    shared, slices = _repack(inputs)
    x = np.ascontiguousarray(inputs["x"], dtype=np.float32).astype(
        __import__("ml_dtypes").bfloat16
    )

    in_maps = []
    for c in range(NCORES):
        sl = slice(BPC * c, BPC * (c + 1))
        in_maps.append({"x2": x[sl], **shared, **slices[c]})

    res = run_bass_kernel_spmd(nc, in_maps, list(range(NCORES))).results
    return np.concatenate([r["yout"] for r in res], axis=0)
